# revision 1
# baseline (speedup 1.0000x reference)
"""Contrastive pairwise-margin loss on 8 Trainium2 NeuronCores.

loss = sum_{i,j} [ R_ij * d_ij + (1-R_ij) * relu(0.5 - d_ij) ] / (N*(N-1)*2)
with d_ij = ||x_i - x_j||^2 and R_ij = [t_i == t_j].

Strategy:
- Host sorts rows by class (the double sum is permutation invariant), so all
  same-class pairs fall inside 512-wide diagonal blocks plus a 128x128 corner
  at each block boundary (requires max class size <= 128; checked, with an
  exact host-side fallback for any leftover pairs).
- Rewrite: loss_ij = relu(m - d_ij) + R_ij * (d_ij - relu(m - d_ij)).
  The first term is computed for ALL pairs; the R-masked second term only on
  the near-diagonal class band.
- The 16x16 grid of 512x512 blocks (upper triangle: 136 blocks) is split
  across 8 cores, 17 blocks each (stripes k and 15-k), off-diagonal blocks
  counted twice. Gram operands are fp8e4m3 (validated: 6e-06 rel error on the
  loss); per [128,512] tile one of two balanced engine paths:
    * ACT path: 2 fp8 gram matmuls + 1 bf16 augmented matmul give
      p = 2*x_i.x_j - sq_j in PSUM (sq_j via 2 augmented K-rows:
      -sq_hi, -sq_lo); ScalarE computes relu(p + (0.5 - sq_i)) with
      per-partition bias and accumulates the per-row sum (accum_out).
    * DVE path (max-trick): only the 2 gram matmuls; VectorE computes
      (p + (0.5 - sq_i)) max bf16(sq_j) with accum_out; since
      relu(z - s) = max(z, s) - s, the host subtracts the exact
      128 * sum_j bf16(sq_j) per tile afterwards.
- Diagonal blocks additionally compute R = onehot_i @ onehot_j^T on the
  TensorEngine over the 384/256-wide class band; ScalarE copies R to SBUF and
  two fused VectorE ops accumulate sum(R*d), sum(R*a).
- Device returns per-tile partial sums [128, 68] + [128, 20]; host applies
  block weights / max-trick constants and reduces in float64.
"""

import os
import sys

for _p in ("/opt/trn_rl_repo", "/root/.axon_site/_ro/trn_rl_repo"):
    if os.path.isdir(_p) and _p not in sys.path:
        sys.path.insert(0, _p)

from contextlib import ExitStack

import ml_dtypes
import numpy as np

import concourse.bass as bass  # noqa: F401
import concourse.mybir as mybir
from concourse import bacc, bass_utils
from concourse.tile import TileContext

BF16 = ml_dtypes.bfloat16
FP8 = ml_dtypes.float8_e4m3
MARGIN = 0.5
N = 8192
D = 256
P = 128
BLK = 512          # block edge (rows/cols)
NBLK = N // BLK    # 16 stripes
NCORES = 8
NBLOCKS = 17       # blocks per core
NTILES = NBLOCKS * 4   # [128,512] tiles per core
NCORR = 20             # correction accum cols (2 ops x (8 diag tiles + 2 corners))
BW = 4 * BLK           # packed big-operand width per block: la0|la1|rb0|rb1

# DMA chunking of the 17 blocks (first chunks small so compute starts early)
CHUNKS = [1, 2, 3, 4, 4, 3]
CHUNK_OF = {}
_c0 = 0
for _g, _n in enumerate(CHUNKS):
    for _b in range(_c0, _c0 + _n):
        CHUNK_OF[_b] = (_g, _b - _c0)
    _c0 += _n

# tiles whose main relu+reduce runs on the DVE max-trick path (no aug matmul);
# correction tiles (b<2) and corner tiles (t=11,15) stay on ACT since their
# DVE budget is taken by the fused correction ops.
_FREE = [t for t in range(NTILES) if t >= 8 and t not in (11, 15) and t < 64]
DVE_TILES = frozenset(_FREE[i] for i in range(len(_FREE)) if i % 3 < 2)  # 36

# class-band slice within a diagonal block, per m-tile index
CORR_SLICE = {0: (0, 256), 1: (0, 384), 2: (128, 384), 3: (256, 256)}

_COMPILED = None       # cached Bacc program
LAST_RESULTS = None    # BassKernelResults of the last run


def _build_program():
    nc = bacc.Bacc("TRN2", target_bir_lowering=False, debug=False,
                   num_devices=NCORES)
    f8 = mybir.dt.float8e4
    bf = mybir.dt.bfloat16
    f32 = mybir.dt.float32
    W = NBLOCKS * BLK  # 8704 packed columns

    big = nc.dram_tensor("big", [P, NBLOCKS * BW], f8, kind="ExternalInput")
    sqb = nc.dram_tensor("sqb", [P, W], bf, kind="ExternalInput")
    la2 = nc.dram_tensor("la2", [2, W], bf, kind="ExternalInput")
    rb2 = nc.dram_tensor("rb2", [2, W], bf, kind="ExternalInput")
    oti = nc.dram_tensor("oti", [P, 2 * BLK], f8, kind="ExternalInput")
    otc = nc.dram_tensor("otc", [P, 2 * P], f8, kind="ExternalInput")
    bias_d = nc.dram_tensor("bias", [P, NTILES], f32, kind="ExternalInput")
    sqi_d = nc.dram_tensor("sqi", [P, 8], f32, kind="ExternalInput")
    acc_d = nc.dram_tensor("acc", [P, NTILES], f32, kind="ExternalOutput")
    cacc_d = nc.dram_tensor("cacc", [P, NCORR], f32, kind="ExternalOutput")

    Relu = mybir.ActivationFunctionType.Relu
    Alu = mybir.AluOpType

    with TileContext(nc) as tc, ExitStack() as ctx:
        sb = ctx.enter_context(tc.tile_pool(name="sb", bufs=1))
        apool = ctx.enter_context(tc.tile_pool(name="apool", bufs=6))
        rpool = ctx.enter_context(tc.tile_pool(name="rpool", bufs=2))
        scpool = ctx.enter_context(tc.tile_pool(name="scpool", bufs=3))
        pp = ctx.enter_context(tc.tile_pool(name="pp", bufs=5, space="PSUM"))
        rp = ctx.enter_context(tc.tile_pool(name="rp", bufs=2, space="PSUM"))

        la2_t = sb.tile([2, W], bf)
        rb2_t = sb.tile([2, W], bf)
        oti_t = sb.tile([P, 2 * BLK], f8)
        otc_t = sb.tile([P, 2 * P], f8)
        bias_t = sb.tile([P, NTILES], f32)
        sqi_t = sb.tile([P, 8], f32)
        acc_t = sb.tile([P, NTILES], f32)
        cacc_t = sb.tile([P, NCORR], f32)

        # warm the ACT table set while DMAs ramp (hides LoadActFuncSet)
        warm = apool.tile([P, 1], f32, tag="warm")
        nc.vector.memset(warm[:], 0.0)
        nc.scalar.activation(warm[:], warm[:],
                             mybir.ActivationFunctionType.Relu)

        # small tensors first so they never gate the pipeline
        for t_, d_ in ((bias_t, bias_d), (sqi_t, sqi_d), (oti_t, oti),
                       (otc_t, otc), (la2_t, la2), (rb2_t, rb2)):
            nc.sync.dma_start(t_[:], d_[:])

        # big fp8 operands + bf16(sq_j) thresholds, chunked
        big_g, sqb_g = [], []
        c0 = 0
        for g, nb in enumerate(CHUNKS):
            bt = sb.tile([P, nb * BW], f8, tag=f"big{g}")
            nc.sync.dma_start(bt[:], big[:, c0 * BW:(c0 + nb) * BW])
            big_g.append(bt)
            st = sb.tile([P, nb * BLK], bf, tag=f"sqb{g}")
            nc.sync.dma_start(st[:], sqb[:, c0 * BLK:(c0 + nb) * BLK])
            sqb_g.append(st)
            c0 += nb

        def ops(b):
            # 3D double-row APs: [128, 2, .] over the packed la0|la1 / rb0|rb1
            g, off = CHUNK_OF[b]
            base = off * BW
            lhs3 = big_g[g][:, base:base + 2 * BLK].rearrange(
                "p (s m) -> p s m", s=2)
            rhs3 = big_g[g][:, base + 2 * BLK:base + 4 * BLK].rearrange(
                "p (s n) -> p s n", s=2)
            return (lambda mi: lhs3[:, :, mi * P:(mi + 1) * P],
                    rhs3,
                    sqb_g[g][:, off * BLK:(off + 1) * BLK])

        def corr_ops(p_ap, a_ap, r_sb, sq_col, out0, out1, w):
            # out0 += sum_j (p - sq_i)*R = -sum R*d ; out1 += sum_j a*R
            sc0 = scpool.tile([P, BLK], f32, tag="sc0")
            sc1 = scpool.tile([P, BLK], f32, tag="sc1")
            nc.vector.scalar_tensor_tensor(
                sc0[:, :w], p_ap, sqi_t[:, sq_col:sq_col + 1], r_sb,
                op0=Alu.subtract, op1=Alu.mult,
                accum_out=cacc_t[:, out0:out0 + 1])
            nc.vector.scalar_tensor_tensor(
                sc1[:, :w], a_ap, 0.0, r_sb,
                op0=Alu.add, op1=Alu.mult,
                accum_out=cacc_t[:, out1:out1 + 1])

        for t in range(NTILES):
            b, mi = divmod(t, 4)
            lhs3, rhs3, sqbs = ops(b)
            lo = mi * P
            dve_path = t in DVE_TILES
            p_t = pp.tile([P, BLK], mybir.dt.float32, tag="p")
            nc.tensor.matmul(p_t[:], lhs3(mi), rhs3,
                             start=True, stop=dve_path,
                             perf_mode=mybir.MatmulPerfMode.DoubleRow)
            if not dve_path:
                glo = b * BLK + mi * P
                nc.tensor.matmul(p_t[:], la2_t[:2, glo:glo + P],
                                 rb2_t[:2, b * BLK:(b + 1) * BLK],
                                 start=False, stop=True)

            if dve_path:
                # sum_j relu(z - s) = sum_j max(z, s) - sum_j s  (host const)
                a_t = apool.tile([P, BLK], bf, tag="adve")
                nc.vector.scalar_tensor_tensor(
                    a_t[:], p_t[:], bias_t[:, t:t + 1], sqbs,
                    op0=Alu.add, op1=Alu.max,
                    accum_out=acc_t[:, t:t + 1])
                continue

            a_t = apool.tile([P, BLK], bf, tag="a")
            nc.scalar.activation(a_t[:], p_t[:], Relu,
                                 bias=bias_t[:, t:t + 1], scale=1.0,
                                 accum_out=acc_t[:, t:t + 1])

            if b < 2:
                # diagonal block: R over the class band of this m-tile
                o, w = CORR_SLICE[mi]
                r_ps = rp.tile([P, BLK], mybir.dt.float32, tag="r")
                nc.tensor.matmul(r_ps[:, :w],
                                 oti_t[:, b * BLK + lo:b * BLK + lo + P],
                                 oti_t[:, b * BLK + o:b * BLK + o + w],
                                 start=True, stop=True)
                r_sb = rpool.tile([P, BLK], bf, tag="rs")
                nc.scalar.copy(r_sb[:, :w], r_ps[:, :w])
                ci = b * 4 + mi
                corr_ops(p_t[:, o:o + w], a_t[:, o:o + w], r_sb[:, :w],
                         ci, 2 * ci, 2 * ci + 1, w)
            elif b in (2, 3) and mi == 3:
                # corner: first 128 cols of the block, last m-tile rows
                c = b - 2
                r_ps = rp.tile([P, P], mybir.dt.float32, tag="r")
                nc.tensor.matmul(r_ps[:], oti_t[:, c * BLK + 384:c * BLK + BLK],
                                 otc_t[:, c * P:(c + 1) * P],
                                 start=True, stop=True)
                r_sb = rpool.tile([P, P], bf, tag="rcs")
                nc.scalar.copy(r_sb[:], r_ps[:])
                sq_col = c * 4 + 3
                corr_ops(p_t[:, 0:P], a_t[:, 0:P], r_sb[:],
                         sq_col, 16 + 2 * c, 17 + 2 * c, P)

        nc.sync.dma_start(acc_d[:], acc_t[:])
        nc.sync.dma_start(cacc_d[:], cacc_t[:])

    nc.compile()
    return nc


def _get_program():
    global _COMPILED
    if _COMPILED is None:
        _COMPILED = _build_program()
    return _COMPILED


def _core_blocks(k):
    """17 (row, col) blocks for core k; first two diagonal, next two carry
    the boundary corners (corner one-hot zeroed for the filler block)."""
    ra, rb = k, NBLK - 1 - k
    blocks_a = [(ra, c) for c in range(ra, NBLK)]
    blocks_b = [(rb, c) for c in range(rb, NBLK)]
    allb = set(blocks_a + blocks_b)
    diag = [(ra, ra), (rb, rb)]
    corn = [(ra, ra + 1)]
    corn_b = (rb, rb + 1)
    has_corn_b = corn_b in allb
    if has_corn_b:
        corn.append(corn_b)
    rest = sorted(allb - set(diag) - set(corn))
    if not has_corn_b:
        corn.append(rest.pop(0))  # filler block; its corner one-hot is zeroed
    order = diag + corn + rest
    assert len(order) == NBLOCKS
    return order, has_corn_b


def kernel(inputs: np.ndarray, target: np.ndarray) -> np.ndarray:
    global LAST_RESULTS
    x = np.asarray(inputs, dtype=np.float32)
    t = np.asarray(target).astype(np.int64)
    assert x.shape == (N, D) and t.shape == (N,)

    perm = np.argsort(t, kind="stable")
    xs = x[perm]
    ts = t[perm]

    sq64 = (xs.astype(np.float64) ** 2).sum(axis=1)
    sq = sq64.astype(np.float32)
    sq_hi = sq.astype(BF16)
    sq_lo = (sq - sq_hi.astype(np.float32)).astype(BF16)
    sqb_row = sq.astype(BF16)                       # bf16(sq_j) for max-trick
    sqb_f64 = sqb_row.astype(np.float64)

    lhs0 = (2.0 * xs[:, :128]).astype(FP8).T.copy()       # [128, N]
    lhs1 = (2.0 * xs[:, 128:]).astype(FP8).T.copy()
    rhs0 = xs[:, :128].astype(FP8).T.copy()
    rhs1 = xs[:, 128:].astype(FP8).T.copy()
    rhs2 = np.stack([-sq_hi, -sq_lo]).astype(BF16)        # [2, N]
    lhs2 = np.ones((2, N), dtype=BF16)
    sqb_full = np.broadcast_to(sqb_row, (P, N))

    onehot = np.zeros((P, N), dtype=FP8)
    onehot[ts, np.arange(N)] = 1

    nclasses = int(ts.max()) + 1
    counts = np.bincount(ts, minlength=nclasses)
    leftover_pairs = counts.max() > P  # exact host fallback, ~never taken

    bias_all = (MARGIN - sq).astype(np.float32)

    in_maps = []
    weights = []
    sconsts = []   # per-core, per-tile max-trick constants (128 * sum_j s'_j)
    for k in range(NCORES):
        order, has_corn_b = _core_blocks(k)
        W = NBLOCKS * BLK
        bigm = np.empty((P, NBLOCKS * BW), FP8)
        la2 = np.empty((2, W), BF16)
        rb2 = np.empty((2, W), BF16)
        sqbm = np.empty((P, W), BF16)
        bias = np.empty((P, NTILES), np.float32)
        sconst = np.zeros(NTILES)
        for bidx, (r, c) in enumerate(order):
            rsl = slice(r * BLK, (r + 1) * BLK)
            csl = slice(c * BLK, (c + 1) * BLK)
            base = bidx * BW
            bigm[:, base:base + BLK] = lhs0[:, rsl]
            bigm[:, base + BLK:base + 2 * BLK] = lhs1[:, rsl]
            bigm[:, base + 2 * BLK:base + 3 * BLK] = rhs0[:, csl]
            bigm[:, base + 3 * BLK:base + 4 * BLK] = rhs1[:, csl]
            dst = slice(bidx * BLK, (bidx + 1) * BLK)
            la2[:, dst] = lhs2[:, rsl]
            rb2[:, dst] = rhs2[:, csl]
            sqbm[:, dst] = sqb_full[:, csl]
            sblock = float(sqb_f64[csl].sum())
            for mi in range(4):
                rows = slice(r * BLK + mi * P, r * BLK + (mi + 1) * P)
                tt = bidx * 4 + mi
                bias[:, tt] = bias_all[rows]
                if tt in DVE_TILES:
                    sconst[tt] = P * sblock
        ra, rbr = order[0][0], order[1][0]
        oti = np.concatenate([onehot[:, ra * BLK:(ra + 1) * BLK],
                              onehot[:, rbr * BLK:(rbr + 1) * BLK]], axis=1)
        otc = np.zeros((P, 2 * P), FP8)
        otc[:, 0:P] = onehot[:, (ra + 1) * BLK:(ra + 1) * BLK + P]
        if has_corn_b:
            otc[:, P:2 * P] = onehot[:, (rbr + 1) * BLK:(rbr + 1) * BLK + P]
        sqi = np.empty((P, 8), np.float32)
        for s, r in enumerate((ra, rbr)):
            for mi in range(4):
                rows = slice(r * BLK + mi * P, r * BLK + (mi + 1) * P)
                sqi[:, s * 4 + mi] = sq[rows]
        in_maps.append({"big": bigm, "sqb": sqbm, "la2": la2, "rb2": rb2,
                        "oti": oti, "otc": otc, "bias": bias, "sqi": sqi})
        weights.append(np.array([1.0 if (r == c) else 2.0
                                 for (r, c) in order]))
        sconsts.append(sconst)

    nc = _get_program()
    res = bass_utils.run_bass_kernel_spmd(
        nc, in_maps, core_ids=list(range(NCORES)))
    LAST_RESULTS = res

    total = 0.0
    for k in range(NCORES):
        out = res.results[k]
        acc = out["acc"].astype(np.float64)    # [128, 68]
        cacc = out["cacc"].astype(np.float64)  # [128, 20]
        w = np.repeat(weights[k], 4)           # per tile
        tile_sums = acc.sum(axis=0) - sconsts[k]   # undo max-trick shift
        total += float((tile_sums * w).sum())
        # diagonal-block corrections (weight 1): sum R*d - sum R*a
        neg_rd = cacc[:, 0:16:2].sum()
        ra_ = cacc[:, 1:16:2].sum()
        total += (-neg_rd) - ra_
        # corner corrections (weight 2)
        neg_rd_c = cacc[:, 16::2].sum()
        ra_c = cacc[:, 17::2].sum()
        total += 2.0 * ((-neg_rd_c) - ra_c)

    if leftover_pairs:
        # exact fp64 host add for same-class pairs not covered by the
        # class-band + corner regions (only if some class has > 128 rows)
        starts = np.concatenate([[0], np.cumsum(counts)])
        for c in range(nclasses):
            lo, hi = starts[c], starts[c + 1]
            if hi - lo <= P:
                continue
            idx = np.arange(lo, hi)
            ii, jj = np.meshgrid(idx, idx, indexing="ij")
            mi_i = (ii % BLK) // P
            band = np.zeros(ii.shape, bool)
            for mi, (o, wd) in CORR_SLICE.items():
                band |= ((mi_i == mi) & (ii // BLK == jj // BLK) &
                         (jj % BLK >= o) & (jj % BLK < o + wd))
            corner = ((jj // BLK == ii // BLK + 1) &
                      (ii % BLK >= BLK - P) & (jj % BLK < P)) | \
                     ((ii // BLK == jj // BLK + 1) &
                      (jj % BLK >= BLK - P) & (ii % BLK < P))
            m = ~(band | corner)
            if m.any():
                xi = xs[ii[m]].astype(np.float64)
                xj = xs[jj[m]].astype(np.float64)
                dd = ((xi - xj) ** 2).sum(axis=1)
                total += float((dd - np.maximum(MARGIN - dd, 0.0)).sum())

    loss = total / (N * (N - 1.0) * 2.0)
    return np.float32(loss)



# revision 5
# speedup vs baseline: 1.4666x; 1.4666x over previous
"""Contrastive pairwise-margin loss on 8 Trainium2 NeuronCores.

loss = sum_{i,j} [ R_ij * d_ij + (1-R_ij) * relu(0.5 - d_ij) ] / (N*(N-1)*2)
with d_ij = ||x_i - x_j||^2 and R_ij = [t_i == t_j].

Decomposition (host rows sorted by class):
  loss_sum = sum_{i!=j} relu(m - d_ij)  +  sum_{same-class, i!=j} [d - relu(m-d)]
The second term is exact fp64 on the host (O(sum n_c^2 * D), tiny). The device
computes only the uniform all-pairs relu sum:
  relu(m - d_ij) = 2 * relu(a_ij),  a_ij = g_ij + h_j + c_i
  g = x_i.x_j (fp8 gram), h_j = -sq_j/2, c_i = (m - sq_i)/2.
Per [128,512] tile: 1 fp8 DoubleRow gram matmul + 1 fp8 DoubleRow "aug"
matmul (Ki=2: rows 1*h_hi, 1*h_lo, c_hi*1, c_lo*1) accumulate a into PSUM;
relu+row-sum is one fused op per 2-bank PSUM group, alternating between
VectorE (tensor_scalar max-0 / add-reduce) and ScalarE (activation Relu with
accum_out). 16x16 upper-tri blocks split 17 per core; lhs/rhs share one
1x-scaled fp8 operand. Host: weights (diag 2, off-diag 4 via relu(2a)=2relu(a)
and block symmetry), exact diagonal-entry emulation, same-class fp64 term.
"""

import os
import sys

for _p in ("/opt/trn_rl_repo", "/root/.axon_site/_ro/trn_rl_repo"):
    if os.path.isdir(_p) and _p not in sys.path:
        sys.path.insert(0, _p)

from contextlib import ExitStack

import ml_dtypes
import numpy as np

import concourse.bass as bass  # noqa: F401
import concourse.mybir as mybir
from concourse import bacc, bass_utils
from concourse.tile import TileContext

FP8 = ml_dtypes.float8_e4m3
MARGIN = 0.5
N = 8192
D = 256
P = 128
BLK = 512
NBLK = N // BLK        # 16 stripes
NCORES = 8
NBLOCKS = 17           # blocks per core
NGROUPS = NBLOCKS * 2  # relu ops per core (2 banks x 2 tiles each)

# DMA chunking of the 17 blocks (first chunks small so compute starts early)
CHUNKS = [1, 1, 2, 3, 4, 3, 3]
CHUNK_OF = {}
_c0 = 0
for _g, _n in enumerate(CHUNKS):
    for _b in range(_c0, _c0 + _n):
        CHUNK_OF[_b] = (_g, _b - _c0)
    _c0 += _n

_COMPILED = None
LAST_RESULTS = None


def _build_program():
    nc = bacc.Bacc("TRN2", target_bir_lowering=False, debug=False,
                   num_devices=NCORES)
    f8 = mybir.dt.float8e4
    bf = mybir.dt.bfloat16
    f32 = mybir.dt.float32
    Alu = mybir.AluOpType
    Relu = mybir.ActivationFunctionType.Relu

    # per block: lhs [128,2,512] | rhs [128,2,512] packed as [128, 4, 512]
    ops_d = nc.dram_tensor("ops", [P, NBLOCKS * 4, BLK], f8,
                           kind="ExternalInput")
    # per block: [ki=2, (ones, cL, hR, ones), 512]
    aug_d = nc.dram_tensor("aug", [2, NBLOCKS * 4, BLK], f8,
                           kind="ExternalInput")
    acc_d = nc.dram_tensor("acc", [P, NGROUPS], f32, kind="ExternalOutput")

    with TileContext(nc) as tc, ExitStack() as ctx:
        sb = ctx.enter_context(tc.tile_pool(name="sb", bufs=1))
        wpool = ctx.enter_context(tc.tile_pool(name="wpool", bufs=2))
        dscr = ctx.enter_context(tc.tile_pool(name="dscr", bufs=2))
        ascr = ctx.enter_context(tc.tile_pool(name="ascr", bufs=2))
        pp = ctx.enter_context(tc.tile_pool(name="pp", bufs=4, space="PSUM"))

        aug_t = sb.tile([2, NBLOCKS * 4, BLK], f8)
        acc_t = sb.tile([P, NGROUPS], f32)

        # warm the ACT table set while DMAs ramp (hides LoadActFuncSet)
        warm = wpool.tile([P, 1], f32, tag="warm")
        nc.vector.memset(warm[:], 0.0)
        nc.scalar.activation(warm[:], warm[:], Relu)

        nc.sync.dma_start(aug_t[:], aug_d[:])

        ops_g = []
        c0 = 0
        for g, nb in enumerate(CHUNKS):
            bt = sb.tile([P, nb * 4, BLK], f8, tag=f"ops{g}")
            nc.sync.dma_start(
                bt[:], ops_d[:, c0 * 4:(c0 + nb) * 4, :])
            ops_g.append(bt)
            c0 += nb

        def block_aps(b):
            g, off = CHUNK_OF[b]
            bt = ops_g[g]
            lhs3 = bt[:, off * 4:off * 4 + 2, :]       # [128, 2, 512]
            rhs3 = bt[:, off * 4 + 2:off * 4 + 4, :]   # [128, 2, 512]
            augL = aug_t[:, b * 4:b * 4 + 2, :]        # [2, 2, 512]
            augR = aug_t[:, b * 4 + 2:b * 4 + 4, :]    # [2, 2, 512]
            return lhs3, rhs3, augL, augR

        DR = mybir.MatmulPerfMode.DoubleRow
        for grp in range(NGROUPS):
            b, half = divmod(grp, 2)
            lhs3, rhs3, augL, augR = block_aps(b)
            p_t = pp.tile([P, 2 * BLK], f32, tag="p")
            for t in range(2):
                mi = 2 * half + t
                sl = slice(t * BLK, (t + 1) * BLK)
                nc.tensor.matmul(p_t[:, sl],
                                 lhs3[:, :, mi * P:(mi + 1) * P], rhs3,
                                 start=True, stop=False, perf_mode=DR)
                nc.tensor.matmul(p_t[:, sl],
                                 augL[:, :, mi * P:(mi + 1) * P], augR,
                                 start=False, stop=True, perf_mode=DR)
            if grp % 2 == 0:
                scr = dscr.tile([P, 2 * BLK], bf, tag="ds")
                nc.vector.tensor_scalar(scr[:], p_t[:], 0.0, 0.0,
                                        op0=Alu.max, op1=Alu.add,
                                        accum_out=acc_t[:, grp:grp + 1])
            else:
                scr = ascr.tile([P, 2 * BLK], bf, tag="as")
                nc.scalar.activation(scr[:], p_t[:], Relu,
                                     accum_out=acc_t[:, grp:grp + 1])

        nc.sync.dma_start(acc_d[:], acc_t[:])

    nc.compile()
    return nc


def _get_program():
    global _COMPILED
    if _COMPILED is None:
        _COMPILED = _build_program()
    return _COMPILED


def _core_blocks(k):
    """17 (row, col) upper-tri blocks for core k; 2 diagonal blocks first."""
    ra, rb = k, NBLK - 1 - k
    order = [(ra, ra), (rb, rb)]
    order += [(ra, c) for c in range(ra + 1, NBLK)]
    order += [(rb, c) for c in range(rb + 1, NBLK)]
    assert len(order) == NBLOCKS
    return order


def kernel(inputs: np.ndarray, target: np.ndarray) -> np.ndarray:
    global LAST_RESULTS
    x = np.asarray(inputs, dtype=np.float32)
    t = np.asarray(target).astype(np.int64)
    assert x.shape == (N, D) and t.shape == (N,)

    perm = np.argsort(t, kind="stable")
    xs = x[perm]
    ts = t[perm]

    x8 = xs.astype(FP8)                       # [N, 256]
    x8f = x8.astype(np.float32)
    sq = (xs.astype(np.float64) ** 2).sum(axis=1).astype(np.float32)

    h = (-0.5 * sq).astype(np.float32)                      # -sq_j / 2
    c = (0.5 * (MARGIN - sq)).astype(np.float32)            # (m - sq_i)/2
    h_hi = h.astype(FP8)
    h_lo = (h - h_hi.astype(np.float32)).astype(FP8)
    c_hi = c.astype(FP8)
    c_lo = (c - c_hi.astype(np.float32)).astype(FP8)

    # packed operand [128, 2, N]: xop[p, s, n] = x8[n, s*128 + p]
    xop = np.ascontiguousarray(x8.T.reshape(2, P, N).transpose(1, 0, 2))

    in_maps = []
    for k in range(NCORES):
        order = _core_blocks(k)
        ops = np.empty((P, NBLOCKS * 4, BLK), FP8)
        aug = np.zeros((2, NBLOCKS * 4, BLK), FP8)
        for j, (r, cb) in enumerate(order):
            rsl = slice(r * BLK, (r + 1) * BLK)
            csl = slice(cb * BLK, (cb + 1) * BLK)
            ops[:, 4 * j:4 * j + 2, :] = xop[:, :, rsl]
            ops[:, 4 * j + 2:4 * j + 4, :] = xop[:, :, csl]
            # aug lhs: [ki, 0, :]=1 ; [ki, 1, m]=c_hi/c_lo of stripe r
            aug[:, 4 * j, :] = 1.0
            aug[0, 4 * j + 1, :] = c_hi[rsl]
            aug[1, 4 * j + 1, :] = c_lo[rsl]
            # aug rhs: [ki, 2, n]=h_hi/h_lo of stripe cb ; [ki, 3, :]=1
            aug[0, 4 * j + 2, :] = h_hi[csl]
            aug[1, 4 * j + 2, :] = h_lo[csl]
            aug[:, 4 * j + 3, :] = 1.0
        in_maps.append({"ops": ops, "aug": aug})

    nc = _get_program()
    res = bass_utils.run_bass_kernel_spmd(
        nc, in_maps, core_ids=list(range(NCORES)))
    LAST_RESULTS = res

    # device sum: groups 0..3 are the two diag blocks (weight 2),
    # groups 4..33 off-diag (weight 2 for symmetry * 2 for relu(2a))
    total = 0.0
    for k in range(NCORES):
        acc = res.results[k]["acc"].astype(np.float64)   # [128, 34]
        col = acc.sum(axis=0)
        total += 2.0 * col[:4].sum() + 4.0 * col[4:].sum()

    # exact diagonal-entry removal (device computed i==j with weight 2)
    g_ii = (x8f * x8f).sum(axis=1, dtype=np.float32)
    a_ii = (g_ii
            + (h_hi.astype(np.float32) + h_lo.astype(np.float32))
            + (c_hi.astype(np.float32) + c_lo.astype(np.float32)))
    total -= 2.0 * np.maximum(a_ii, 0.0).astype(np.float64).sum()

    # exact same-class term in fp64: sum_{same, i != j} [d - relu(m - d)]
    sq64 = (xs.astype(np.float64) ** 2).sum(axis=1)
    nclasses = int(ts.max()) + 1
    counts = np.bincount(ts, minlength=nclasses)
    starts = np.concatenate([[0], np.cumsum(counts)])
    for cc in range(nclasses):
        lo, hi = starts[cc], starts[cc + 1]
        if hi - lo < 2:
            continue
        Xc = xs[lo:hi].astype(np.float64)
        sqc = sq64[lo:hi]
        dm = sqc[:, None] + sqc[None, :] - 2.0 * (Xc @ Xc.T)
        np.fill_diagonal(dm, np.nan)
        dsum = np.nansum(dm)
        rsum = np.nansum(np.maximum(MARGIN - dm, 0.0))
        total += dsum - rsum

    loss = total / (N * (N - 1.0) * 2.0)
    return np.float32(loss)


# revision 11
# speedup vs baseline: 1.6063x; 1.0953x over previous
"""Contrastive pairwise-margin loss on 8 Trainium2 NeuronCores.

loss = sum_{i,j} [ R_ij * d_ij + (1-R_ij) * relu(0.5 - d_ij) ] / (N*(N-1)*2)
with d_ij = ||x_i - x_j||^2 and R_ij = [t_i == t_j].

Decomposition (host rows sorted by class):
  loss_sum = sum_{i!=j} relu(m - d_ij)  +  sum_{same-class, i!=j} [d - relu(m-d)]
The second term is exact fp64 on the host (O(sum n_c^2 * D), tiny). The device
computes only the uniform all-pairs relu sum over the upper-triangle blocks:
  relu(m - d_ij) = 2 * relu(a_ij),  a_ij = g_ij + h_j + c_i
  g = x_i.x_j (fp8 gram), h_j = -sq_j/2, c_i = (m - sq_i)/2.
Per [128,<=512] tile: 1 fp8 DoubleRow gram matmul + 1 fp8 DoubleRow "aug"
matmul (Ki=2: rows 1*h_hi, 1*h_lo, c_hi*1, c_lo*1) accumulate a into PSUM;
relu+row-sum is one fused op per 2-bank PSUM group, split between VectorE
(tensor_scalar max-0 / add-reduce) and ScalarE (activation Relu + accum_out)
by a static cost balance. Diagonal blocks compute only their upper triangle
(per-mi column offsets) and reuse the lhs operand as rhs. Host: everything
x4 (2 from relu(2a)=2relu(a), 2 from block symmetry), minus 4*relu(a_ii)
(device diagonal entries, emulated exactly), plus the exact same-class term.
"""

import os
import sys

for _p in ("/opt/trn_rl_repo", "/root/.axon_site/_ro/trn_rl_repo"):
    if os.path.isdir(_p) and _p not in sys.path:
        sys.path.insert(0, _p)

from contextlib import ExitStack

import ml_dtypes
import numpy as np

import concourse.bass as bass  # noqa: F401
import concourse.mybir as mybir
from concourse import bacc, bass_utils
from concourse.tile import TileContext

FP8 = ml_dtypes.float8_e4m3
MARGIN = 0.5
N = 8192
D = 256
P = 128
BLK = 512
NBLK = N // BLK        # 16 stripes
NCORES = 8
NBLOCKS = 17           # blocks per core (2 diag + 15 off-diag)
NGROUPS = NBLOCKS * 2  # relu ops per core
NWARM = 30             # PE p-state warm-up matmuls

# operand slots: 0,1 = diag blocks (lhs==rhs); off-diag block i (pos 2..16)
# has lhs slot 2+2*(i-2), rhs slot 3+2*(i-2)  -> 2 + 15*2 = 32 slots
NSLOTS = 2 + 15 * 2

# DMA chunking in slot units (first chunks small so compute starts early)
SLOT_CHUNKS = [1, 1, 2, 4, 6, 8, 6, 4]
assert sum(SLOT_CHUNKS) == NSLOTS

# group table: (block_pos, [(mi, rhs_off, width, psum_off)...], width, wgt)
# diag blocks: strict-upper tile slices (weight 4) + the four diagonal
# 128x128 sub-squares packed into one 512-wide group (weight 2).
GROUPS = []
for _pos in range(2):
    GROUPS.append((_pos, [(0, 128, 384, 0), (1, 256, 256, 384)], 640, 4.0))
    GROUPS.append((_pos, [(2, 384, 128, 0)], 128, 4.0))
    GROUPS.append((_pos, [(mi, mi * P, P, mi * P) for mi in range(4)],
                   512, 2.0))
for _pos in range(2, NBLOCKS):
    GROUPS.append((_pos, [(0, 0, BLK, 0), (1, 0, BLK, BLK)], 1024, 4.0))
    GROUPS.append((_pos, [(2, 0, BLK, 0), (3, 0, BLK, BLK)], 1024, 4.0))

# static engine split balanced by modeled op cost
def _op_cost(width, eng):
    if eng == "dve":
        return (width + 120) * 1.0417
    return (width + 222) * 0.8333 + 187

def _assign_engines():
    dve_t = act_t = 0.0
    out = []
    for _, _, w, _wgt in GROUPS:
        cd, ca = _op_cost(w, "dve"), _op_cost(w, "act")
        if dve_t + cd <= act_t + ca:
            out.append("dve")
            dve_t += cd
        else:
            out.append("act")
            act_t += ca
    return out

ENGINES = _assign_engines()
DVE_COLS = [i for i, e in enumerate(ENGINES) if e == "dve"]
ACC_COL = {}
for _i, _g in enumerate(DVE_COLS):
    ACC_COL[_g] = ("dve", _i)
_ACT_COLS = [i for i, e in enumerate(ENGINES) if e == "act"]
for _i, _g in enumerate(_ACT_COLS):
    ACC_COL[_g] = ("act", _i)
N_DVE = len(DVE_COLS)
N_ACT = len(_ACT_COLS)
W_DVE = np.array([GROUPS[g][3] for g in DVE_COLS])
W_ACT = np.array([GROUPS[g][3] for g in _ACT_COLS])

_CHUNK_OF_SLOT = {}
_c0 = 0
for _g, _n in enumerate(SLOT_CHUNKS):
    for _s in range(_c0, _c0 + _n):
        _CHUNK_OF_SLOT[_s] = (_g, _s - _c0)
    _c0 += _n

def _pos_slots(pos):
    """(lhs_slot, rhs_slot) for block position."""
    if pos < 2:
        return pos, pos
    return 2 + 2 * (pos - 2), 3 + 2 * (pos - 2)

_COMPILED = None
LAST_RESULTS = None


def _build_program():
    nc = bacc.Bacc("TRN2", target_bir_lowering=False, debug=False,
                   num_devices=NCORES)
    f8 = mybir.dt.float8e4
    bf = mybir.dt.bfloat16
    f32 = mybir.dt.float32
    Alu = mybir.AluOpType
    Relu = mybir.ActivationFunctionType.Relu

    ops_d = nc.dram_tensor("ops", [P, NSLOTS * 2, BLK], f8,
                           kind="ExternalInput")
    aug_d = nc.dram_tensor("aug", [2, NBLOCKS * 4, BLK], f8,
                           kind="ExternalInput")
    accd_d = nc.dram_tensor("accd", [P, N_DVE], f32, kind="ExternalOutput")
    acca_d = nc.dram_tensor("acca", [P, N_ACT], f32, kind="ExternalOutput")

    with TileContext(nc) as tc, ExitStack() as ctx:
        sb = ctx.enter_context(tc.tile_pool(name="sb", bufs=1))
        wpool = ctx.enter_context(tc.tile_pool(name="wpool", bufs=2))
        dscr = ctx.enter_context(tc.tile_pool(name="dscr", bufs=2))
        ascr = ctx.enter_context(tc.tile_pool(name="ascr", bufs=2))
        pp = ctx.enter_context(tc.tile_pool(name="pp", bufs=4, space="PSUM"))

        aug_t = sb.tile([2, NBLOCKS * 4, BLK], f8)
        accd_t = sb.tile([P, N_DVE], f32)
        acca_t = sb.tile([P, N_ACT], f32)

        # warm the ACT table set while DMAs ramp (hides LoadActFuncSet)
        warm = wpool.tile([P, 1], f32, tag="warm")
        nc.vector.memset(warm[:], 0.0)
        nc.scalar.activation(warm[:], warm[:], Relu)

        # PE p-state warm-up: chain of cheap matmuls on a zeroed operand
        wop = wpool.tile([P, 2, 256], f8, tag="wop")
        nc.vector.memset(wop[:], 0.0)
        wp = pp.tile([P, 2 * BLK], f32, tag="p")
        DR = mybir.MatmulPerfMode.DoubleRow
        for _ in range(NWARM):
            nc.tensor.matmul(wp[:, 0:256], wop[:, :, 0:P], wop[:],
                             start=True, stop=True, perf_mode=DR)

        nc.sync.dma_start(aug_t[:], aug_d[:])

        slots_g = []
        c0 = 0
        for g, nb in enumerate(SLOT_CHUNKS):
            bt = sb.tile([P, nb * 2, BLK], f8, tag=f"ops{g}")
            nc.sync.dma_start(bt[:], ops_d[:, c0 * 2:(c0 + nb) * 2, :])
            slots_g.append((c0, bt))
            c0 += nb

        def slot_ap(s):
            g, off = _CHUNK_OF_SLOT[s]
            bt = slots_g[g][1]
            return bt[:, off * 2:off * 2 + 2, :]   # [128, 2, 512]

        for grp, (pos, parts, width, _wgt) in enumerate(GROUPS):
            ls, rs = _pos_slots(pos)
            lhs3 = slot_ap(ls)
            rhs3 = slot_ap(rs)
            augL = aug_t[:, pos * 4:pos * 4 + 2, :]
            augR = aug_t[:, pos * 4 + 2:pos * 4 + 4, :]
            p_t = pp.tile([P, 2 * BLK], f32, tag="p")
            for mi, off, w, pcol in parts:
                sl = slice(pcol, pcol + w)
                nc.tensor.matmul(p_t[:, sl],
                                 lhs3[:, :, mi * P:(mi + 1) * P],
                                 rhs3[:, :, off:off + w],
                                 start=True, stop=False, perf_mode=DR)
                nc.tensor.matmul(p_t[:, sl],
                                 augL[:, :, mi * P:(mi + 1) * P],
                                 augR[:, :, off:off + w],
                                 start=False, stop=True, perf_mode=DR)
            eng, acol = ACC_COL[grp]
            if eng == "dve":
                scr = dscr.tile([P, 2 * BLK], bf, tag="ds")
                nc.vector.tensor_scalar(scr[:, :width], p_t[:, :width],
                                        0.0, 0.0, op0=Alu.max, op1=Alu.add,
                                        accum_out=accd_t[:, acol:acol + 1])
            else:
                scr = ascr.tile([P, 2 * BLK], bf, tag="as")
                nc.scalar.activation(scr[:, :width], p_t[:, :width], Relu,
                                     accum_out=acca_t[:, acol:acol + 1])

        nc.sync.dma_start(accd_d[:], accd_t[:])
        nc.sync.dma_start(acca_d[:], acca_t[:])

    nc.compile()
    return nc


def _get_program():
    global _COMPILED
    if _COMPILED is None:
        _COMPILED = _build_program()
    return _COMPILED


def _core_blocks(k):
    """17 (row, col) upper-tri blocks for core k; the 2 diagonal first."""
    ra, rb = k, NBLK - 1 - k
    order = [(ra, ra), (rb, rb)]
    order += [(ra, c) for c in range(ra + 1, NBLK)]
    order += [(rb, c) for c in range(rb + 1, NBLK)]
    assert len(order) == NBLOCKS
    return order


def kernel(inputs: np.ndarray, target: np.ndarray) -> np.ndarray:
    global LAST_RESULTS
    x = np.asarray(inputs, dtype=np.float32)
    t = np.asarray(target).astype(np.int64)
    assert x.shape == (N, D) and t.shape == (N,)

    perm = np.argsort(t, kind="stable")
    xs = x[perm]
    ts = t[perm]

    x8 = xs.astype(FP8)                       # [N, 256]
    x8f = x8.astype(np.float32)
    sq = (xs.astype(np.float64) ** 2).sum(axis=1).astype(np.float32)

    h = (-0.5 * sq).astype(np.float32)                      # -sq_j / 2
    c = (0.5 * (MARGIN - sq)).astype(np.float32)            # (m - sq_i)/2
    h_hi = h.astype(FP8)
    h_lo = (h - h_hi.astype(np.float32)).astype(FP8)
    c_hi = c.astype(FP8)
    c_lo = (c - c_hi.astype(np.float32)).astype(FP8)

    # packed operand [128, 2, N]: xop[p, s, n] = x8[n, s*128 + p]
    xop = np.ascontiguousarray(x8.T.reshape(2, P, N).transpose(1, 0, 2))

    in_maps = []
    for k in range(NCORES):
        order = _core_blocks(k)
        ops = np.empty((P, NSLOTS * 2, BLK), FP8)
        aug = np.zeros((2, NBLOCKS * 4, BLK), FP8)
        for pos, (r, cb) in enumerate(order):
            rsl = slice(r * BLK, (r + 1) * BLK)
            csl = slice(cb * BLK, (cb + 1) * BLK)
            ls, rs = _pos_slots(pos)
            ops[:, 2 * ls:2 * ls + 2, :] = xop[:, :, rsl]
            if rs != ls:
                ops[:, 2 * rs:2 * rs + 2, :] = xop[:, :, csl]
            aug[:, 4 * pos, :] = 1.0
            aug[0, 4 * pos + 1, :] = c_hi[rsl]
            aug[1, 4 * pos + 1, :] = c_lo[rsl]
            aug[0, 4 * pos + 2, :] = h_hi[csl]
            aug[1, 4 * pos + 2, :] = h_lo[csl]
            aug[:, 4 * pos + 3, :] = 1.0
        in_maps.append({"ops": ops, "aug": aug})

    nc = _get_program()
    res = bass_utils.run_bass_kernel_spmd(
        nc, in_maps, core_ids=list(range(NCORES)))
    LAST_RESULTS = res

    total = 0.0
    for k in range(NCORES):
        accd = res.results[k]["accd"].astype(np.float64)
        acca = res.results[k]["acca"].astype(np.float64)
        total += (accd.sum(axis=0) * W_DVE).sum()
        total += (acca.sum(axis=0) * W_ACT).sum()

    # exact diagonal-entry removal (device computed i==j in the weight-2
    # diagonal sub-square groups)
    g_ii = (x8f * x8f).sum(axis=1, dtype=np.float32)
    a_ii = (g_ii
            + (h_hi.astype(np.float32) + h_lo.astype(np.float32))
            + (c_hi.astype(np.float32) + c_lo.astype(np.float32)))
    total -= 2.0 * np.maximum(a_ii, 0.0).astype(np.float64).sum()

    # exact same-class term in fp64: sum_{same, i != j} [d - relu(m - d)]
    sq64 = (xs.astype(np.float64) ** 2).sum(axis=1)
    nclasses = int(ts.max()) + 1
    counts = np.bincount(ts, minlength=nclasses)
    starts = np.concatenate([[0], np.cumsum(counts)])
    for cc in range(nclasses):
        lo, hi = starts[cc], starts[cc + 1]
        if hi - lo < 2:
            continue
        Xc = xs[lo:hi].astype(np.float64)
        sqc = sq64[lo:hi]
        dm = sqc[:, None] + sqc[None, :] - 2.0 * (Xc @ Xc.T)
        np.fill_diagonal(dm, np.nan)
        total += np.nansum(dm) - np.nansum(np.maximum(MARGIN - dm, 0.0))

    loss = total / (N * (N - 1.0) * 2.0)
    return np.float32(loss)


# revision 17
# speedup vs baseline: 1.6855x; 1.0493x over previous
"""Contrastive pairwise-margin loss on 8 Trainium2 NeuronCores.

loss = sum_{i,j} [ R_ij * d_ij + (1-R_ij) * relu(0.5 - d_ij) ] / (N*(N-1)*2)
with d_ij = ||x_i - x_j||^2 and R_ij = [t_i == t_j].

Decomposition (host rows sorted by class):
  loss_sum = sum_{i!=j} relu(m - d_ij)  +  sum_{same-class, i!=j} [d - relu(m-d)]
The second term is exact fp64 on the host (O(sum n_c^2 * D), tiny). The device
computes only the uniform all-pairs relu sum over the upper-triangle blocks:
  relu(m - d_ij) = 2 * relu(a_ij),  a_ij = g_ij + h_j + c_i
  g = x_i.x_j (fp8 gram), h_j = -sq_j/2, c_i = (m - sq_i)/2.
Per [128,<=512] tile: 1 fp8 DoubleRow gram matmul + 1 fp8 DoubleRow "aug"
matmul (Ki=2: rows 1*h_hi, 1*h_lo, c_hi*1, c_lo*1) accumulate a into PSUM;
relu+row-sum is one fused op per 2-bank PSUM group, split between VectorE
(tensor_scalar max-0 / add-reduce) and ScalarE (activation Relu + accum_out)
by a static cost balance. Diagonal blocks compute only their upper triangle
(per-mi column offsets) and reuse the lhs operand as rhs. Host: everything
x4 (2 from relu(2a)=2relu(a), 2 from block symmetry), minus 4*relu(a_ii)
(device diagonal entries, emulated exactly), plus the exact same-class term.
"""

import os
import sys

for _p in ("/opt/trn_rl_repo", "/root/.axon_site/_ro/trn_rl_repo"):
    if os.path.isdir(_p) and _p not in sys.path:
        sys.path.insert(0, _p)

from contextlib import ExitStack

import ml_dtypes
import numpy as np

import concourse.bass as bass  # noqa: F401
import concourse.mybir as mybir
from concourse import bacc, bass_utils
from concourse.tile import TileContext

FP8 = ml_dtypes.float8_e4m3
MARGIN = 0.5
N = 8192
D = 256
P = 128
BLK = 512
NBLK = N // BLK        # 16 stripes
NCORES = 8
NBLOCKS = 17           # blocks per core (2 diag + 15 off-diag)
NGROUPS = NBLOCKS * 2  # relu ops per core
NWARM = 20             # PE p-state warm-up matmuls

# operand slots: 0,1 = diag blocks (lhs==rhs); off-diag block i (pos 2..16)
# has lhs slot 2+2*(i-2), rhs slot 3+2*(i-2)  -> 2 + 15*2 = 32 slots
NSLOTS = 2 + 15 * 2

# DMA chunking in slot units (first chunks small so compute starts early)
SLOT_CHUNKS = [1, 1, 2, 4, 6, 8, 6, 4]
assert sum(SLOT_CHUNKS) == NSLOTS

# group table: (block_pos, [(mi, rhs_off, width, psum_off)...], width, wgt)
# diag blocks: strict-upper tile slices (weight 4) + the four diagonal
# 128x128 sub-squares packed into one 512-wide group (weight 2).
GROUPS = []
for _pos in range(2):
    GROUPS.append((_pos, [(0, 128, 384, 0)], 384, 4.0))
    GROUPS.append((_pos, [(1, 256, 256, 0), (2, 384, 128, 256)], 384, 4.0))
    GROUPS.append((_pos, [(mi, mi * P, P, mi * P) for mi in range(4)],
                   512, 2.0))
for _pos in range(2, NBLOCKS):
    GROUPS.append((_pos, [(0, 0, BLK, 0), (1, 0, BLK, BLK)], 1024, 4.0))
    GROUPS.append((_pos, [(2, 0, BLK, 0), (3, 0, BLK, BLK)], 1024, 4.0))

# static engine split balanced by modeled op cost
def _op_cost(width, eng):
    if eng == "dve":
        return (width + 120) * 1.0417
    return (width + 222) * 0.8333 + 187

def _assign_engines():
    dve_t = act_t = 0.0
    out = []
    for _, _, w, _wgt in GROUPS:
        cd, ca = _op_cost(w, "dve"), _op_cost(w, "act")
        if dve_t + cd <= act_t + ca:
            out.append("dve")
            dve_t += cd
        else:
            out.append("act")
            act_t += ca
    return out

ENGINES = _assign_engines()
DVE_COLS = [i for i, e in enumerate(ENGINES) if e == "dve"]
ACC_COL = {}
for _i, _g in enumerate(DVE_COLS):
    ACC_COL[_g] = ("dve", _i)
_ACT_COLS = [i for i, e in enumerate(ENGINES) if e == "act"]
for _i, _g in enumerate(_ACT_COLS):
    ACC_COL[_g] = ("act", _i)
N_DVE = len(DVE_COLS)
N_ACT = len(_ACT_COLS)
W_DVE = np.array([GROUPS[g][3] for g in DVE_COLS])
W_ACT = np.array([GROUPS[g][3] for g in _ACT_COLS])

_CHUNK_OF_SLOT = {}
_c0 = 0
for _g, _n in enumerate(SLOT_CHUNKS):
    for _s in range(_c0, _c0 + _n):
        _CHUNK_OF_SLOT[_s] = (_g, _s - _c0)
    _c0 += _n

def _pos_slots(pos):
    """(lhs_slot, rhs_slot) for block position."""
    if pos < 2:
        return pos, pos
    return 2 + 2 * (pos - 2), 3 + 2 * (pos - 2)

_COMPILED = None
LAST_RESULTS = None


def _build_program():
    nc = bacc.Bacc("TRN2", target_bir_lowering=False, debug=False,
                   num_devices=NCORES)
    f8 = mybir.dt.float8e4
    bf = mybir.dt.bfloat16
    f32 = mybir.dt.float32
    Alu = mybir.AluOpType
    Relu = mybir.ActivationFunctionType.Relu

    ops_d = nc.dram_tensor("ops", [P, NSLOTS * 2, BLK], f8,
                           kind="ExternalInput")
    aug_d = nc.dram_tensor("aug", [2, NBLOCKS * 4, BLK], f8,
                           kind="ExternalInput")
    accd_d = nc.dram_tensor("accd", [P, N_DVE], f32, kind="ExternalOutput")
    acca_d = nc.dram_tensor("acca", [P, N_ACT], f32, kind="ExternalOutput")

    with TileContext(nc) as tc, ExitStack() as ctx:
        sb = ctx.enter_context(tc.tile_pool(name="sb", bufs=1))
        wpool = ctx.enter_context(tc.tile_pool(name="wpool", bufs=2))
        pp = ctx.enter_context(tc.tile_pool(name="pp", bufs=4, space="PSUM"))

        aug_t = sb.tile([2, NBLOCKS * 4, BLK], f8)
        accd_t = sb.tile([P, N_DVE], f32)
        acca_t = sb.tile([P, N_ACT], f32)

        # warm the ACT table set while DMAs ramp (hides LoadActFuncSet)
        warm = wpool.tile([P, 1], f32, tag="warm")
        nc.gpsimd.memset(warm[:], 0.0)
        nc.scalar.activation(warm[:], warm[:], Relu)

        # PE p-state warm-up: chain of cheap matmuls on a zeroed operand
        wop = wpool.tile([P, 2, 256], f8, tag="wop")
        nc.gpsimd.memset(wop[:], 0.0)
        wp = pp.tile([P, 2 * BLK], f32, tag="p")
        DR = mybir.MatmulPerfMode.DoubleRow
        for _ in range(NWARM):
            nc.tensor.matmul(wp[:, 0:256], wop[:, :, 0:P], wop[:],
                             start=True, stop=True, perf_mode=DR)

        nc.sync.dma_start(aug_t[:], aug_d[:])

        slots_g = []
        c0 = 0
        for g, nb in enumerate(SLOT_CHUNKS):
            bt = sb.tile([P, nb * 2, BLK], f8, tag=f"ops{g}")
            nc.sync.dma_start(bt[:], ops_d[:, c0 * 2:(c0 + nb) * 2, :])
            slots_g.append((c0, bt))
            c0 += nb

        def slot_ap(s):
            g, off = _CHUNK_OF_SLOT[s]
            bt = slots_g[g][1]
            return bt[:, off * 2:off * 2 + 2, :]   # [128, 2, 512]

        for grp, (pos, parts, width, _wgt) in enumerate(GROUPS):
            ls, rs = _pos_slots(pos)
            lhs3 = slot_ap(ls)
            rhs3 = slot_ap(rs)
            augL = aug_t[:, pos * 4:pos * 4 + 2, :]
            augR = aug_t[:, pos * 4 + 2:pos * 4 + 4, :]
            p_t = pp.tile([P, 2 * BLK], f32, tag="p")
            for mi, off, w, pcol in parts:
                sl = slice(pcol, pcol + w)
                nc.tensor.matmul(p_t[:, sl],
                                 lhs3[:, :, mi * P:(mi + 1) * P],
                                 rhs3[:, :, off:off + w],
                                 start=True, stop=False, perf_mode=DR)
                nc.tensor.matmul(p_t[:, sl],
                                 augL[:, :, mi * P:(mi + 1) * P],
                                 augR[:, :, off:off + w],
                                 start=False, stop=True, perf_mode=DR)
            eng, acol = ACC_COL[grp]
            if eng == "dve":
                nc.vector.tensor_scalar(p_t[:, :width], p_t[:, :width],
                                        0.0, 0.0, op0=Alu.max, op1=Alu.add,
                                        accum_out=accd_t[:, acol:acol + 1])
            else:
                nc.scalar.activation(p_t[:, :width], p_t[:, :width], Relu,
                                     accum_out=acca_t[:, acol:acol + 1])

        nc.sync.dma_start(accd_d[:], accd_t[:])
        nc.gpsimd.dma_start(acca_d[:], acca_t[:])

    nc.compile()
    return nc


def _get_program():
    global _COMPILED
    if _COMPILED is None:
        _COMPILED = _build_program()
    return _COMPILED


def _core_blocks(k):
    """17 (row, col) upper-tri blocks for core k; the 2 diagonal first."""
    ra, rb = k, NBLK - 1 - k
    order = [(ra, ra), (rb, rb)]
    order += [(ra, c) for c in range(ra + 1, NBLK)]
    order += [(rb, c) for c in range(rb + 1, NBLK)]
    assert len(order) == NBLOCKS
    return order


def kernel(inputs: np.ndarray, target: np.ndarray) -> np.ndarray:
    global LAST_RESULTS
    x = np.asarray(inputs, dtype=np.float32)
    t = np.asarray(target).astype(np.int64)
    assert x.shape == (N, D) and t.shape == (N,)

    perm = np.argsort(t, kind="stable")
    xs = x[perm]
    ts = t[perm]

    x8 = xs.astype(FP8)                       # [N, 256]
    x8f = x8.astype(np.float32)
    sq = (xs.astype(np.float64) ** 2).sum(axis=1).astype(np.float32)

    h = (-0.5 * sq).astype(np.float32)                      # -sq_j / 2
    c = (0.5 * (MARGIN - sq)).astype(np.float32)            # (m - sq_i)/2
    h_hi = h.astype(FP8)
    h_lo = (h - h_hi.astype(np.float32)).astype(FP8)
    c_hi = c.astype(FP8)
    c_lo = (c - c_hi.astype(np.float32)).astype(FP8)

    # packed operand [128, 2, N]: xop[p, s, n] = x8[n, s*128 + p]
    xop = np.ascontiguousarray(x8.T.reshape(2, P, N).transpose(1, 0, 2))

    in_maps = []
    for k in range(NCORES):
        order = _core_blocks(k)
        ops = np.empty((P, NSLOTS * 2, BLK), FP8)
        aug = np.zeros((2, NBLOCKS * 4, BLK), FP8)
        for pos, (r, cb) in enumerate(order):
            rsl = slice(r * BLK, (r + 1) * BLK)
            csl = slice(cb * BLK, (cb + 1) * BLK)
            ls, rs = _pos_slots(pos)
            ops[:, 2 * ls:2 * ls + 2, :] = xop[:, :, rsl]
            if rs != ls:
                ops[:, 2 * rs:2 * rs + 2, :] = xop[:, :, csl]
            aug[:, 4 * pos, :] = 1.0
            aug[0, 4 * pos + 1, :] = c_hi[rsl]
            aug[1, 4 * pos + 1, :] = c_lo[rsl]
            aug[0, 4 * pos + 2, :] = h_hi[csl]
            aug[1, 4 * pos + 2, :] = h_lo[csl]
            aug[:, 4 * pos + 3, :] = 1.0
        in_maps.append({"ops": ops, "aug": aug})

    nc = _get_program()
    res = bass_utils.run_bass_kernel_spmd(
        nc, in_maps, core_ids=list(range(NCORES)))
    LAST_RESULTS = res

    total = 0.0
    for k in range(NCORES):
        accd = res.results[k]["accd"].astype(np.float64)
        acca = res.results[k]["acca"].astype(np.float64)
        total += (accd.sum(axis=0) * W_DVE).sum()
        total += (acca.sum(axis=0) * W_ACT).sum()

    # exact diagonal-entry removal (device computed i==j in the weight-2
    # diagonal sub-square groups)
    g_ii = (x8f * x8f).sum(axis=1, dtype=np.float32)
    a_ii = (g_ii
            + (h_hi.astype(np.float32) + h_lo.astype(np.float32))
            + (c_hi.astype(np.float32) + c_lo.astype(np.float32)))
    total -= 2.0 * np.maximum(a_ii, 0.0).astype(np.float64).sum()

    # exact same-class term in fp64: sum_{same, i != j} [d - relu(m - d)]
    sq64 = (xs.astype(np.float64) ** 2).sum(axis=1)
    nclasses = int(ts.max()) + 1
    counts = np.bincount(ts, minlength=nclasses)
    starts = np.concatenate([[0], np.cumsum(counts)])
    for cc in range(nclasses):
        lo, hi = starts[cc], starts[cc + 1]
        if hi - lo < 2:
            continue
        Xc = xs[lo:hi].astype(np.float64)
        sqc = sq64[lo:hi]
        dm = sqc[:, None] + sqc[None, :] - 2.0 * (Xc @ Xc.T)
        np.fill_diagonal(dm, np.nan)
        total += np.nansum(dm) - np.nansum(np.maximum(MARGIN - dm, 0.0))

    loss = total / (N * (N - 1.0) * 2.0)
    return np.float32(loss)
